# revision 12
# baseline (speedup 1.0000x reference)
"""Trainium2 Bass kernel for the Clebsch-Gordan tensor-product layer.

Math (per batch element b, per triple (l1, l2, l)):
    out[b, p, t1, t2] = sum_{m1, m2} CG[p, m1*M2+m2] * a[b, m1, t1] * c[b, m2, t2]
with a = f_{l1}, c = f_{l2} complex (last dim = re/im), CG real.

Factorization used on-device (two matmul stages):
    S[b, (l,p), m1, t2] = sum_{m2} CG[p, m1*M2+m2] * c[b, m2, t2]     (stage A)
    out[b, p, t1, t2]   = sum_{m1} a[b, m1, t1] * S[b, (l,p), m1, t2] (stage B)

Sharding: pure data parallelism, batch 128 -> 16 per core x 8 cores.

Stage A: contraction m2 on partitions, CG-derived weights (host packed),
  psum rows = (m1-major, lp-minor) chunks of <=128, free = (comp,b,t2)=768.
Redistribute: SBUF->SBUF DMA moves S to [32*bgrp + m1, (q, comp, j, t2)].
Stage B: 16 concurrent tile_position matmuls (4 b-groups x 4 b's), contraction
  m1, psum [(j, t1), (q, t2)], complex handled by 2-matmul psum accumulation
  with host-packed {ar, ai, -ai} weights.
Output: drains interleave re/im; one big DMA per (pair, l) slab.
"""

import os
import sys
import numpy as np

_TRN_REPO = "/opt/trn_rl_repo"
for _p in (_TRN_REPO, os.path.join(_TRN_REPO, "concourse")):
    if _p not in sys.path:
        sys.path.insert(0, _p)

LMAX = 5
TAU = 24
BATCH = 128
NCORES = 8
BC = BATCH // NCORES  # 16 batch elements per core

# ---------------------------------------------------------------- metadata ---


def _triples():
    out = []
    for l1 in range(LMAX + 1):
        for l2 in range(l1 + 1):
            for l in range(abs(l1 - l2), min(l1 + l2, LMAX) + 1):
                out.append((l1, l2, l))
    return out


TRIPLES = _triples()
TRIP_IDX = {t: i for i, t in enumerate(TRIPLES)}

# number of triples contributing to each output l
N_L = [sum(1 for (_, _, l) in TRIPLES if l == lo) for lo in range(LMAX + 1)]

# slab index of triple within its output-l concatenation
SLAB = {}
_ctr = [0] * (LMAX + 1)
for t in TRIPLES:
    SLAB[t] = _ctr[t[2]]
    _ctr[t[2]] += 1


class Pair:
    def __init__(self, l1, l2):
        self.l1, self.l2 = l1, l2
        self.m1, self.m2 = 2 * l1 + 1, 2 * l2 + 1
        self.Ls = [l for l in range(l1 - l2, min(l1 + l2, LMAX) + 1)]
        self.P = sum(2 * l + 1 for l in self.Ls)
        # q axis enumerates (l, p): l ascending (triple order), p within l
        self.qoff = {}
        o = 0
        for l in self.Ls:
            self.qoff[l] = o
            o += 2 * l + 1
        # stage-A chunks: groups of q (lp) values, rows = m1 * G <= 128
        gmax = 128 // self.m1
        self.chunksA = []  # (q0, G)
        q0 = 0
        while q0 < self.P:
            g = min(gmax, self.P - q0)
            self.chunksA.append((q0, g))
            q0 += g
        # stage-B q chunks: N = Gq*24 <= 504 (one psum bank)
        self.chunksB = []
        q0 = 0
        while q0 < self.P:
            g = min(21, self.P - q0)
            self.chunksB.append((q0, g))
            q0 += g


PAIRS = [Pair(l1, l2) for l1 in range(LMAX + 1) for l2 in range(l1 + 1)]

# cw column layout: per pair, per chunk: col = base + mu*G + g
_cwoff = {}
_NW = 0
for _pr in PAIRS:
    for _ci, (_q0, _g) in enumerate(_pr.chunksA):
        _cwoff[(_pr.l1, _pr.l2, _ci)] = _NW
        _NW += _pr.m1 * _g
NW = _NW

F_CS = 2 * BC * TAU  # 768, per-l2 stage-A rhs block (comp, b, t2)
F_AW = 4 * 3 * 32  # 384, per-l1 stage-B weight block (j, {ar,ai,-ai}, t1pad32)

_SUBSET = None  # debug hook: list of (l1,l2) to restrict pairs


def _active_pairs():
    if _SUBSET is None:
        return PAIRS
    return [p for p in PAIRS if (p.l1, p.l2) in _SUBSET]


# ------------------------------------------------------------ host packing ---


def pack_cw(cg_all):
    """Stage-A weights [m2<=11 rows, NW cols] from cg_all [69, 11, 121]."""
    cw = np.zeros((11, NW), dtype=np.float32)
    for pr in PAIRS:
        for ci, (q0, G) in enumerate(pr.chunksA):
            base = _cwoff[(pr.l1, pr.l2, ci)]
            for g in range(G):
                q = q0 + g
                # decode q -> (l, p)
                for l in pr.Ls:
                    if q < pr.qoff[l] + 2 * l + 1:
                        p = q - pr.qoff[l]
                        break
                ti = TRIP_IDX[(pr.l1, pr.l2, l)]
                for mu in range(pr.m1):
                    cw[: pr.m2, base + mu * G + g] = cg_all[
                        ti, p, mu * pr.m2 : mu * pr.m2 + pr.m2
                    ]
    return cw


def pack_cs(fs):
    """Stage-A rhs [11, 6*768]: cs[nu, l2*768 + c*384 + b*24 + t2]."""
    cs = np.zeros((11, 6 * F_CS), dtype=np.float32)
    for l2 in range(LMAX + 1):
        f = fs[l2]  # [BC, 24, 2*l2+1, 2]
        # -> [nu, c, b, t2]
        v = np.transpose(f, (2, 3, 0, 1)).reshape(2 * l2 + 1, F_CS)
        cs[: 2 * l2 + 1, l2 * F_CS : (l2 + 1) * F_CS] = v
    return cs


def pack_aw(fs):
    """Stage-B weights [128, 6*288]: aw[32i+mu, l1*288 + j*72 + c3*24 + t1]."""
    aw = np.zeros((128, 6 * F_AW), dtype=np.float32)
    for l1 in range(LMAX + 1):
        m1 = 2 * l1 + 1
        f = fs[l1]  # [BC, 24, m1, 2]
        for i in range(4):
            for j in range(4):
                b = 4 * i + j
                blk = f[b]  # [24, m1, 2]
                for c3, v in enumerate(
                    (blk[:, :, 0], blk[:, :, 1], -blk[:, :, 1])
                ):
                    # v [t1, mu] -> aw rows 32i+mu, cols ... + c3*32 + t1
                    o = l1 * F_AW + j * 96 + c3 * 32
                    aw[32 * i : 32 * i + m1, o : o + 24] = v.T
    return aw


# ------------------------------------------------------------ bass program ---

_PROG = None  # cached (nc,)


def _build_program():
    import concourse.mybir as mybir
    from concourse import bacc
    import concourse.tile as tile
    from concourse.tile_rust import add_dep_helper

    f32 = mybir.dt.float32
    nc = bacc.Bacc("TRN2", target_bir_lowering=False, debug=False)

    aw_d = nc.dram_tensor("aw", [128, 6 * F_AW], f32, kind="ExternalInput").ap()
    cw_d = nc.dram_tensor("cw", [11, NW], f32, kind="ExternalInput").ap()
    cs_d = nc.dram_tensor("cs", [11, 6 * F_CS], f32, kind="ExternalInput").ap()
    out_d = [
        nc.dram_tensor(
            f"o{l}", [BC, N_L[l] * 576, 2 * l + 1, 2], f32, kind="ExternalOutput"
        ).ap()
        for l in range(LMAX + 1)
    ]

    pairs = _active_pairs()
    Pmax = max(pr.P for pr in pairs)

    with tile.TileContext(nc) as tc:
        with (
            tc.tile_pool(name="const", bufs=1) as cpool,
            tc.tile_pool(name="sdrain", bufs=4) as sdpool,
            tc.tile_pool(name="spool", bufs=2) as spool,
            tc.tile_pool(name="osb", bufs=2) as opool,
            tc.tile_pool(name="psA", bufs=2, space="PSUM") as psA,
            tc.tile_pool(name="psB", bufs=6, space="PSUM") as psB,
        ):
            aw_sb = cpool.tile([128, 6 * F_AW], f32, tag="aw")
            cw_sb = cpool.tile([128, NW], f32, tag="cw")
            cs_sb = cpool.tile([128, 6 * F_CS], f32, tag="cs")
            nc.sync.dma_start(out=aw_sb[:, :], in_=aw_d[:, :])
            nc.sync.dma_start(out=cw_sb[:11, :], in_=cw_d[:, :])
            nc.sync.dma_start(out=cs_sb[:11, :], in_=cs_d[:, :])

            for pr in pairs:
                m1, m2, P = pr.m1, pr.m2, pr.P
                # S tile: [32i+mu, q*192 + c*96 + j*24 + t2]
                S_t = spool.tile([128, 192 * Pmax], f32, tag="S")

                # ---- stage A + redistribute ----
                for ci, (q0, G) in enumerate(pr.chunksA):
                    R = m1 * G
                    base = _cwoff[(pr.l1, pr.l2, ci)]
                    sd = sdpool.tile([128, F_CS], f32, tag="sd")
                    for h in range(2):
                        pa = psA.tile([128, 384], f32, tag="pa", padded_shape=[128, 512])
                        nc.tensor.matmul(
                            pa[:R, :384],
                            lhsT=cw_sb[:m2, base : base + R],
                            rhs=cs_sb[:m2, pr.l2 * F_CS + h * 384 : pr.l2 * F_CS + (h + 1) * 384],
                            start=True,
                            stop=True,
                        )
                        cp = nc.vector.tensor_copy if h == 0 else nc.scalar.copy
                        cp(out=sd[:R, h * 384 : (h + 1) * 384], in_=pa[:R, :384])
                    # redistribute into S_t
                    src = sd[:R, :].rearrange(
                        "r (c bb t) -> r c bb t", c=2, bb=BC, t=TAU
                    )
                    dstv = S_t[:, : 192 * P].rearrange(
                        "mm (q c j t) -> mm q c j t", q=P, c=2, j=4, t=TAU
                    )
                    for i in range(4):
                        nc.sync.dma_start(
                            out=dstv[32 * i : 32 * i + m1, q0 : q0 + G, :, :, :],
                            in_=src[:, :, 4 * i : 4 * i + 4, :],
                        )

                # ---- stage B ----
                # osb free layout: [i, l-region] with l-region = (t2, p, ri)
                # region offset for l: 48 * qoff[l]; region size 48*(2l+1)
                osb = opool.tile([128, 4 * Pmax * 48], f32, tag="osb")
                last_drains = {}  # engine-name -> last drain instruction
                Sv = S_t[:, : 192 * P].rearrange(
                    "mm (q c j t) -> mm q c j t", q=P, c=2, j=4, t=TAU
                )
                for q0, Gq in pr.chunksB:
                    N = Gq * TAU
                    for rc in range(2):  # 0 = real, 1 = imag output
                        pbs = [
                            psB.tile([128, 504], f32, tag="pb", name="pb", padded_shape=[128, 512])
                            for _ in range(4)
                        ]
                        # two accumulation terms: (c3 weight, c S-comp)
                        terms = ((0, 0), (2, 1)) if rc == 0 else ((1, 0), (0, 1))
                        for i in range(4):
                            for j in range(4):
                                for ti, (c3, c) in enumerate(terms):
                                    wof = pr.l1 * F_AW + j * 96 + c3 * 32
                                    nc.tensor.matmul(
                                        pbs[i][32 * j : 32 * j + 32, :N],
                                        lhsT=aw_sb[32 * i : 32 * i + m1, wof : wof + 32],
                                        rhs=Sv[
                                            32 * i : 32 * i + m1, q0 : q0 + Gq, c, j, :
                                        ],
                                        start=(ti == 0),
                                        stop=(ti == 1),
                                        tile_position=(32 * i, 32 * j),
                                    )
                        # drain per (i, l-segment): psum (part, q, t2) ->
                        # osb[part, i*48P + 48*qoff_l + t2*(2l+1)*2 + p*2 + rc]
                        cp = nc.vector.tensor_copy
                        ekey = "dve"
                        for i in range(4):
                            pv = pbs[i][:, :N].rearrange(
                                "pp (q t) -> pp q t", q=Gq, t=TAU
                            )
                            for l in pr.Ls:
                                pl = 2 * l + 1
                                lo = max(q0, pr.qoff[l])
                                hi = min(q0 + Gq, pr.qoff[l] + pl)
                                if lo >= hi:
                                    continue
                                seg = osb[
                                    :, i * 48 * P + 48 * pr.qoff[l] : i * 48 * P + 48 * (pr.qoff[l] + pl)
                                ].rearrange("pp (t p r) -> pp p t r", t=TAU, p=pl, r=2)
                                last_drains[ekey] = cp(
                                    out=seg[:, lo - pr.qoff[l] : hi - pr.qoff[l], :, rc],
                                    in_=pv[:, lo - q0 : hi - q0, :],
                                )

                # ---- output DMAs: one per (pair, l, j) ----
                # SBUF side must keep a SINGLE partition-crossing dim, so
                # partition = t1 (24 rows at base 32j); i rides in free.
                first_dma = True
                for l in pr.Ls:
                    pl = 2 * l + 1
                    s = SLAB[(pr.l1, pr.l2, l)]
                    qo = pr.qoff[l]
                    dvl = out_d[l].rearrange(
                        "(ii jj) (s t1 f) p r -> s jj t1 ii (f p r)",
                        ii=4,
                        jj=4,
                        s=N_L[l],
                        t1=TAU,
                        f=TAU,
                    )[s]
                    for j in range(4):
                        ovj = osb[32 * j : 32 * j + TAU, : 4 * P * 48].rearrange(
                            "tt (i f) -> tt i f", i=4, f=P * 48
                        )[:, :, 48 * qo : 48 * (qo + pl)]
                        dinst = nc.sync.dma_start(out=dvl[j], in_=ovj)
                        # Belt-and-braces: force dep on the last drain for the
                        # first DMA of the pair (queue FIFO covers the rest).
                        if first_dma:
                            first_dma = False
                            add_dep_helper(
                                dinst.ins, last_drains["dve"].ins, sync=True,
                                reason="osb drained",
                            )

    nc.compile()
    return nc


def _get_program():
    global _PROG
    if _PROG is None:
        _PROG = _build_program()
    return _PROG


# ------------------------------------------------------------------ driver ---


def kernel(f0, f1, f2, f3, f4, f5, cg_all):
    from concourse.bass_utils import run_bass_kernel_spmd

    fs_full = [np.asarray(f, dtype=np.float32) for f in (f0, f1, f2, f3, f4, f5)]
    cg = np.asarray(cg_all, dtype=np.float32)

    nc = _get_program()
    cw = pack_cw(cg)
    in_maps = []
    for k in range(NCORES):
        sh = [f[k * BC : (k + 1) * BC] for f in fs_full]
        in_maps.append({"aw": pack_aw(sh), "cw": cw, "cs": pack_cs(sh)})

    res = run_bass_kernel_spmd(nc, in_maps, list(range(NCORES)))
    outs = []
    for l in range(LMAX + 1):
        full = np.concatenate(
            [res.results[k][f"o{l}"] for k in range(NCORES)], axis=0
        )
        outs.append(full.astype(np.float32))
    return tuple(outs)
